# revision 11
# baseline (speedup 1.0000x reference)
"""Trainium2 Bass kernel for nn_BNNConv2d (LSQ 8-bit act quant + 2nd-order
binary-residual weight quant + 3x3 conv, NCHW, pad 1, stride 1) on 8 cores.

Strategy
--------
Data-parallel over batch: 16 images -> 2 per NeuronCore; weights/scalings
replicated.  Per core:

  1. x shard (2,256,64,64) f32 streams to SBUF; |x| partial sums reduce on
     DVE/GpSimd; cross-partition sum via an fp32 ones-matmul; the 8 per-core
     scalars AllReduce to the global sum -> alpha = 2*mean|x|/sqrt(127).
  2. Activations quantize to INTEGER-VALUED bf16 (exact: integers in
     [-128,127]): q = ((x*(1/a) clamped) + 1.5*2^23) - 1.5*2^23 (RNE round
     trick), written into zero-padded [128, 66, 66] images per (image,
     ci-half).  Pipelined DVE -> GpSimd -> ScalarE in row-halves.
  3. Effective weight bw = sign(w)*s1 + sign(w - sign(w)*s1)*s2 built on DVE
     (exact sign via clamp(w*1e38, -1, 1)), cast to bf16 (terms=1), or split
     bw = hi + lo into two bf16 terms (terms=2, near-exact).  bf16 x bf16
     products against integer activations are exact; PSUM accumulates fp32.
     The dominant error vs the jax reference is the fp32 summation-order
     difference in alpha (XLA CPU mean), which flips ~1e-4 of the 16.7M
     quantization roundings -> ~3e-3 absmax rel err for either terms setting.
  4. Conv = 9 shifted matmuls per ci-half per term accumulated in PSUM over
     strips of 8 output rows (N=512 = one PSUM bank), 2 Cout tiles of 128.
  5. Epilogue y = alpha*psum + bias on DVE/ScalarE (alternating), DMA out.

The walrus build in this container only supports ONE sync wait per
instruction; _split_waits() hoists extra waits onto same-engine NOPs.
"""

import sys

for _p in ("/opt/trn_rl_repo", "/root/.axon_site"):
    if _p not in sys.path:
        sys.path.insert(0, _p)

import numpy as np

import concourse.bass as bass
import concourse.mybir as mybir
import concourse.tile as tile

dt = mybir.dt
Alu = mybir.AluOpType
Act = mybir.ActivationFunctionType

N_CORES = 8
B, CIN, H, W = 16, 256, 64, 64
COUT, K = 256, 3
B_LOC = B // N_CORES            # images per core
NCIG = CIN // 128               # ci groups of 128 partitions
NCOG = COUT // 128              # co tiles of 128
HP, WP = H + 2, W + 2           # padded image
ROWS_PER_STRIP = 8              # 8 rows x 64 cols = 512 = one PSUM bank
NSTRIP = H // ROWS_PER_STRIP
NTAP = K * K
TERMS = 1                       # bf16 weight terms (1: fast, 2: hi+lo split)

# round-to-int magic: v + 1.5*2^23 stays in [2^23, 2^24) for v in [-128,127],
# where the fp32 ulp is exactly 1.0 -> the add rounds v to the nearest int (RNE)
ROUND_C = float(2 ** 23 + 2 ** 22)
INV_SQRT_QP = float(np.float32(1.0) / np.float32(np.sqrt(127.0)))


def _split_waits(nc, max_waits=1):
    """Split >max_waits sync waits per instruction onto same-engine NOPs."""
    for bb in nc.main_func.blocks:
        il = bb.instructions
        i = 0
        while i < len(il):
            ins = il[i]
            si = ins.sync_info
            if si is not None and si.on_wait and len(si.on_wait) > max_waits:
                waits = list(si.on_wait)
                extra, keep = waits[:-max_waits], waits[-max_waits:]
                k = 0
                while extra:
                    chunk, extra = extra[:max_waits], extra[max_waits:]
                    nop = mybir.InstNoOp(
                        name=f"{ins.name}-wsplit{k}", ins=[], outs=[])
                    nop.engine = ins.engine
                    nop.sync_info = mybir.SyncInfo(on_wait=chunk, on_update=[])
                    il.insert(i, nop)
                    i += 1
                    k += 1
                ins.sync_info = mybir.SyncInfo(
                    on_wait=keep, on_update=list(si.on_update))
            i += 1


def _build(terms=TERMS, debug_taps=False):
    nc = bass.Bass("TRN2", target_bir_lowering=False, debug=False,
                   num_devices=N_CORES)

    x_d = nc.dram_tensor("x", [B_LOC, CIN, H, W], dt.float32,
                         kind="ExternalInput")
    # weight pre-laid-out host-side as [ci, tap*256 + co]
    w_d = nc.dram_tensor("wt", [CIN, NTAP * COUT], dt.float32,
                         kind="ExternalInput")
    # s1/s2 replicated host-side to [128, tap*256+co]
    s1_d = nc.dram_tensor("s1f", [128, NTAP * COUT], dt.float32,
                          kind="ExternalInput")
    s2_d = nc.dram_tensor("s2f", [128, NTAP * COUT], dt.float32,
                          kind="ExternalInput")
    b_d = nc.dram_tensor("bias", [COUT, 1], dt.float32, kind="ExternalInput")
    y_d = nc.dram_tensor("y", [B_LOC, COUT, H, W], dt.float32,
                         kind="ExternalOutput")
    cc_in = nc.dram_tensor("cc_in", [1, 8], dt.float32)
    cc_out = nc.dram_tensor("cc_out", [1, 8], dt.float32)
    if debug_taps:
        dbg_a = nc.dram_tensor("dbg_alpha", [1, 2], dt.float32,
                               kind="ExternalOutput")
        dbg_xq = nc.dram_tensor("dbg_xq", [128, HP, WP], dt.float32,
                                kind="ExternalOutput")

    FREE_W = NTAP * COUT  # 2304
    HHALF = H // 2

    with tile.TileContext(nc, num_cores=N_CORES) as tc:
        with tc.tile_pool(name="persist", bufs=1) as pp, \
             tc.tile_pool(name="xs", bufs=3) as xsp, \
             tc.tile_pool(name="wtmp", bufs=1) as wp, \
             tc.tile_pool(name="stage", bufs=6) as sp, \
             tc.tile_pool(name="cpsum", bufs=4, space="PSUM") as cps, \
             tc.tile_pool(name="spsum", bufs=1, space="PSUM") as sps, \
             tc.tile_pool(name="dram", bufs=1, space="DRAM") as dp:

            # constants first (cheap, engines idle)
            ones_r = pp.tile([1, 128], dt.float32, tag="ones_r")
            nc.vector.memset(ones_r[:], 1.0)
            ones_c = pp.tile([128, 1], dt.float32, tag="ones_c")
            nc.vector.memset(ones_c[:], 1.0)

            # ------- pass A: stream x, abs-reduce -> AllReduce (critical) ---
            parts = pp.tile([128, B_LOC * NCIG], dt.float32, tag="parts")
            for b in range(B_LOC):
                for g in range(NCIG):
                    t = xsp.tile([128, H * W], dt.float32, tag="xs")
                    nc.sync.dma_start(t[:], x_d[b, g * 128:(g + 1) * 128, :, :])
                    nc.vector.tensor_reduce(
                        parts[:, b * NCIG + g: b * NCIG + g + 1],
                        t[:], op=Alu.add, axis=mybir.AxisListType.X,
                        apply_absolute_value=True)
            partred = pp.tile([128, 1], dt.float32, tag="partred")
            nc.vector.tensor_reduce(partred[:], parts[:], op=Alu.add,
                                    axis=mybir.AxisListType.X)
            stot_ps = sps.tile([1, 1], dt.float32, tag="stot")
            nc.tensor.matmul(stot_ps[:], partred[:], ones_c[:],
                             start=True, stop=True)
            ssb = pp.tile([1, 8], dt.float32, tag="ssb")
            nc.vector.memset(ssb[:], 0.0)
            nc.vector.tensor_copy(ssb[:, 0:1], stot_ps[:])
            nc.sync.dma_start(cc_in[:], ssb[:])
            nc.gpsimd.collective_compute(
                "AllReduce", Alu.add,
                replica_groups=[list(range(N_CORES))],
                ins=[cc_in[:]], outs=[cc_out[:]])
            sglob = pp.tile([1, 8], dt.float32, tag="sglob")
            nc.sync.dma_start(sglob[:], cc_out[:])

            # alpha = (2*mean|x|) / sqrt(127); recip = 1/alpha
            alpha_s = pp.tile([1, 1], dt.float32, tag="alpha_s")
            nc.vector.tensor_scalar(alpha_s[:], sglob[:, 0:1],
                                    float(2.0 ** -23), INV_SQRT_QP,
                                    op0=Alu.mult, op1=Alu.mult)
            recip_s = pp.tile([1, 1], dt.float32, tag="recip_s")
            nc.vector.reciprocal(recip_s[:], alpha_s[:])
            # broadcast both to [128,1] via K=1 fp32 matmuls (exact)
            al_ps = sps.tile([128, 1], dt.float32, tag="al_ps")
            nc.tensor.matmul(al_ps[:], ones_r[:], alpha_s[:],
                             start=True, stop=True)
            alpha_bc = pp.tile([128, 1], dt.float32, tag="alpha_bc")
            nc.vector.tensor_copy(alpha_bc[:], al_ps[:])
            rc_ps = sps.tile([128, 1], dt.float32, tag="rc_ps")
            nc.tensor.matmul(rc_ps[:], ones_r[:], recip_s[:],
                             start=True, stop=True)
            recip_bc = pp.tile([128, 1], dt.float32, tag="recip_bc")
            nc.vector.tensor_copy(recip_bc[:], rc_ps[:])

            # ------- xq padded buffers: zero the borders early ------------
            xq = {}
            for b in range(B_LOC):
                for g in range(NCIG):
                    t = pp.tile([128, HP, WP], dt.bfloat16, tag=f"xq{b}{g}")
                    nc.vector.memset(t[:, 0:1, :], 0.0)
                    nc.vector.memset(t[:, HP - 1:HP, :], 0.0)
                    nc.vector.memset(t[:, 1:HP - 1, 0:1], 0.0)
                    nc.vector.memset(t[:, 1:HP - 1, WP - 1:WP], 0.0)
                    xq[b, g] = t

            # ------- small input DMAs + weight prep (independent of alpha) -
            bias_t = []
            for c in range(NCOG):
                bt = pp.tile([128, 1], dt.float32, tag=f"bias{c}")
                nc.sync.dma_start(bt[:], b_d[c * 128:(c + 1) * 128, :])
                bias_t.append(bt)
            s1_t = pp.tile([128, FREE_W], dt.float32, tag="s1")
            nc.sync.dma_start(s1_t[:], s1_d[:])
            s2_t = pp.tile([128, FREE_W], dt.float32, tag="s2")
            nc.sync.dma_start(s2_t[:], s2_d[:])

            wq = {}   # (g, term) -> [128, 2304] bf16
            for g in range(NCIG):
                Wg = wp.tile([128, FREE_W], dt.float32, tag="w")
                nc.sync.dma_start(Wg[:], w_d[g * 128:(g + 1) * 128, :])
                sgnw = wp.tile([128, FREE_W], dt.float32, tag="sgnw")
                nc.vector.tensor_scalar(sgnw[:], Wg[:], 1e38, -1.0,
                                        op0=Alu.mult, op1=Alu.max)
                nc.vector.tensor_scalar(sgnw[:], sgnw[:], 1.0, None,
                                        op0=Alu.min)
                t1 = wp.tile([128, FREE_W], dt.float32, tag="t1")
                nc.vector.tensor_tensor(t1[:], sgnw[:], s1_t[:], op=Alu.mult)
                res = wp.tile([128, FREE_W], dt.float32, tag="res")
                nc.vector.tensor_tensor(res[:], Wg[:], t1[:], op=Alu.subtract)
                nc.vector.tensor_scalar(res[:], res[:], 1e38, -1.0,
                                        op0=Alu.mult, op1=Alu.max)
                nc.vector.tensor_scalar(res[:], res[:], 1.0, None,
                                        op0=Alu.min)
                nc.vector.tensor_tensor(res[:], res[:], s2_t[:], op=Alu.mult)
                bw = wp.tile([128, FREE_W], dt.float32, tag="bw")
                nc.vector.tensor_tensor(bw[:], t1[:], res[:], op=Alu.add)
                hi = pp.tile([128, FREE_W], dt.bfloat16, tag=f"hi{g}")
                nc.scalar.activation(hi[:], bw[:], Act.Copy)
                wq[g, 0] = hi
                if terms == 2:
                    lo_f = wp.tile([128, FREE_W], dt.float32, tag="sgnw")
                    nc.vector.tensor_tensor(lo_f[:], bw[:], hi[:],
                                            op=Alu.subtract)
                    lo = pp.tile([128, FREE_W], dt.bfloat16, tag=f"lo{g}")
                    nc.scalar.activation(lo[:], lo_f[:], Act.Copy)
                    wq[g, 1] = lo

            # ------- quantize: pass B re-streams x -------------------------
            # Per (b,g), split rows in half; pipeline DVE -> GpSimd -> ACT.
            for b in range(B_LOC):
                for g in range(NCIG):
                    q = xsp.tile([128, H * W], dt.float32, tag="xs")
                    nc.sync.dma_start(q[:], x_d[b, g * 128:(g + 1) * 128, :, :])
                    q3 = q[:].rearrange("p (h w) -> p h w", h=H)
                    for h0 in (0, HHALF):
                        sl = q3[:, h0:h0 + HHALF, :]
                        # q = min(x * (1/a), 127)          (DVE)
                        nc.vector.tensor_scalar(sl, sl, recip_bc[:, 0:1],
                                                127.0, op0=Alu.mult,
                                                op1=Alu.min)
                        # q = max(q, -128) + C  (rounds RNE on f32 write)
                        nc.gpsimd.tensor_scalar(sl, sl, -128.0, ROUND_C,
                                                op0=Alu.max, op1=Alu.add)
                        # interior <- q - C as bf16 (exact small integers)
                        nc.scalar.activation(
                            xq[b, g][:, 1 + h0:1 + h0 + HHALF, 1:WP - 1],
                            sl, Act.Copy, bias=-ROUND_C)

            if debug_taps:
                da = pp.tile([1, 2], dt.float32, tag="dbg_a")
                nc.vector.tensor_copy(da[:, 0:1], alpha_s[:])
                nc.vector.tensor_copy(da[:, 1:2], recip_s[:])
                nc.sync.dma_start(dbg_a[:], da[:])
                dxq = pp.tile([128, HP, WP], dt.float32, tag="dbg_xq")
                nc.vector.tensor_copy(dxq[:], xq[0, 0][:])
                nc.sync.dma_start(dbg_xq[:], dxq[:])

            # ------- conv ---------------------------------------------------
            taps = [(ky, kx) for ky in range(K) for kx in range(K)]
            ep_idx = 0
            for b in range(B_LOC):
                for s in range(NSTRIP):
                    r0 = s * ROWS_PER_STRIP
                    for c in range(NCOG):
                        ps = cps.tile([128, ROWS_PER_STRIP * W], dt.float32,
                                      tag="cps")
                        n_mm = terms * NCIG * NTAP
                        i_mm = 0
                        for term in range(terms):
                            for g in range(NCIG):
                                wqt = wq[g, term]
                                for (ky, kx) in taps:
                                    tap = ky * K + kx
                                    lhsT = wqt[:, tap * COUT + c * 128:
                                               tap * COUT + c * 128 + 128]
                                    rhs = xq[b, g][:, r0 + ky:
                                                   r0 + ky + ROWS_PER_STRIP,
                                                   kx:kx + W]
                                    nc.tensor.matmul(
                                        ps[:], lhsT, rhs,
                                        start=(i_mm == 0),
                                        stop=(i_mm == n_mm - 1))
                                    i_mm += 1
                        out_t = sp.tile([128, ROWS_PER_STRIP * W], dt.float32,
                                        tag="out")
                        # y = alpha*psum + bias; alternate engines (ACT is
                        # ~2.3x slower per op but otherwise idle)
                        if ep_idx % 3 == 2:
                            nc.scalar.activation(out_t[:], ps[:], Act.Identity,
                                                 bias=bias_t[c][:, 0:1],
                                                 scale=alpha_bc[:, 0:1])
                        else:
                            nc.vector.tensor_scalar(out_t[:], ps[:],
                                                    alpha_bc[:, 0:1],
                                                    bias_t[c][:, 0:1],
                                                    op0=Alu.mult, op1=Alu.add)
                        ep_idx += 1
                        nc.sync.dma_start(
                            y_d[b, c * 128:(c + 1) * 128,
                                r0:r0 + ROWS_PER_STRIP, :],
                            out_t[:])

    _split_waits(nc, 1)
    return nc


def _prep_host_inputs(x, weight, bias, scaling_first_order,
                      scaling_second_order):
    """Layout-only host prep: shard x over batch, relayout/replicate weights."""
    w_t = np.ascontiguousarray(
        weight.transpose(1, 2, 3, 0)).reshape(CIN, NTAP * COUT)
    s1 = np.asarray(scaling_first_order, np.float32).reshape(COUT)
    s2 = np.asarray(scaling_second_order, np.float32).reshape(COUT)
    s1f = np.ascontiguousarray(
        np.broadcast_to(np.tile(s1, NTAP), (128, NTAP * COUT)))
    s2f = np.ascontiguousarray(
        np.broadcast_to(np.tile(s2, NTAP), (128, NTAP * COUT)))
    b_r = np.asarray(bias, np.float32).reshape(COUT, 1)
    x = np.asarray(x, np.float32)
    in_maps = []
    for i in range(N_CORES):
        in_maps.append({
            "x": np.ascontiguousarray(x[i * B_LOC:(i + 1) * B_LOC]),
            "wt": w_t, "s1f": s1f, "s2f": s2f, "bias": b_r,
        })
    return in_maps


def _install_ntff_hook():
    import types
    try:
        import antenv.axon_hooks  # noqa: F401
        return
    except ImportError:
        pass
    from trn_agent_boot.trn_boot import _ntff_profile_via_ctypes
    hook = _ntff_profile_via_ctypes('/opt/axon/libaxon_pjrt.so')
    mod = types.ModuleType('antenv.axon_hooks')
    mod.get_axon_ntff_profile_hook = lambda: hook
    mod.set_axon_ntff_profile_hook = lambda h: None
    sys.modules['antenv.axon_hooks'] = mod


def _run(in_maps, terms=TERMS, trace=False, debug_taps=False):
    from concourse.bass_utils import run_bass_kernel_spmd
    if trace:
        _install_ntff_hook()
        from concourse import bass_utils
        bass_utils.upload_artifacts = lambda tmpdir: f"local:{tmpdir}"
    nc = _build(terms, debug_taps=debug_taps)
    return run_bass_kernel_spmd(nc, in_maps, list(range(N_CORES)),
                                trace=trace)


def kernel(x, weight, bias, scaling_first_order, scaling_second_order):
    in_maps = _prep_host_inputs(x, weight, bias, scaling_first_order,
                                scaling_second_order)
    res = _run(in_maps, TERMS, trace=False)
    return np.concatenate([res.results[i]["y"] for i in range(N_CORES)],
                          axis=0)


# revision 12
# speedup vs baseline: 1.7007x; 1.7007x over previous
"""Trainium2 Bass kernel for nn_BNNConv2d (LSQ 8-bit act quant + 2nd-order
binary-residual weight quant + 3x3 conv, NCHW, pad 1, stride 1) on 8 cores.

Strategy
--------
Data-parallel over batch: 16 images -> 2 per NeuronCore; weights/scalings
replicated.  Per core:

  1. x shard (2,256,64,64) f32 streams to SBUF; |x| partial sums reduce on
     DVE/GpSimd; cross-partition sum via an fp32 ones-matmul; the 8 per-core
     scalars AllReduce to the global sum -> alpha = 2*mean|x|/sqrt(127).
  2. Activations quantize to INTEGER-VALUED bf16 (exact: integers in
     [-128,127]): q = ((x*(1/a) clamped) + 1.5*2^23) - 1.5*2^23 (RNE round
     trick), written into zero-padded [128, 66, 66] images per (image,
     ci-half).  Pipelined DVE -> GpSimd -> ScalarE in row-halves.
  3. Effective weight bw = sign(w)*s1 + sign(w - sign(w)*s1)*s2 built on DVE
     (exact sign via clamp(w*1e38, -1, 1)), cast to bf16 (terms=1), or split
     bw = hi + lo into two bf16 terms (terms=2, near-exact).  bf16 x bf16
     products against integer activations are exact; PSUM accumulates fp32.
     The dominant error vs the jax reference is the fp32 summation-order
     difference in alpha (XLA CPU mean), which flips ~1e-4 of the 16.7M
     quantization roundings -> ~3e-3 absmax rel err for either terms setting.
  4. Conv = 9 shifted matmuls per ci-half per term accumulated in PSUM over
     strips of 8 output rows (N=512 = one PSUM bank), 2 Cout tiles of 128.
  5. Epilogue y = alpha*psum + bias on DVE/ScalarE (alternating), DMA out.

The walrus build in this container only supports ONE sync wait per
instruction; _split_waits() hoists extra waits onto same-engine NOPs.
"""

import sys

for _p in ("/opt/trn_rl_repo", "/root/.axon_site"):
    if _p not in sys.path:
        sys.path.insert(0, _p)

import numpy as np

import concourse.bass as bass
import concourse.mybir as mybir
import concourse.tile as tile

dt = mybir.dt
Alu = mybir.AluOpType
Act = mybir.ActivationFunctionType

N_CORES = 8
B, CIN, H, W = 16, 256, 64, 64
COUT, K = 256, 3
B_LOC = B // N_CORES            # images per core
NCIG = CIN // 128               # ci groups of 128 partitions
NCOG = COUT // 128              # co tiles of 128
HP, WP = H + 2, W + 2           # padded image
ROWS_PER_STRIP = 8              # 8 rows x 64 cols = 512 = one PSUM bank
NSTRIP = H // ROWS_PER_STRIP
NTAP = K * K
TERMS = 1                       # bf16 weight terms (1: fast, 2: hi+lo split)

# round-to-int magic: v + 1.5*2^23 stays in [2^23, 2^24) for v in [-128,127],
# where the fp32 ulp is exactly 1.0 -> the add rounds v to the nearest int (RNE)
ROUND_C = float(2 ** 23 + 2 ** 22)
INV_SQRT_QP = float(np.float32(1.0) / np.float32(np.sqrt(127.0)))


def _split_waits(nc, max_waits=1):
    """Split >max_waits sync waits per instruction onto same-engine NOPs."""
    for bb in nc.main_func.blocks:
        il = bb.instructions
        i = 0
        while i < len(il):
            ins = il[i]
            si = ins.sync_info
            if si is not None and si.on_wait and len(si.on_wait) > max_waits:
                waits = list(si.on_wait)
                extra, keep = waits[:-max_waits], waits[-max_waits:]
                k = 0
                while extra:
                    chunk, extra = extra[:max_waits], extra[max_waits:]
                    nop = mybir.InstNoOp(
                        name=f"{ins.name}-wsplit{k}", ins=[], outs=[])
                    nop.engine = ins.engine
                    nop.sync_info = mybir.SyncInfo(on_wait=chunk, on_update=[])
                    il.insert(i, nop)
                    i += 1
                    k += 1
                ins.sync_info = mybir.SyncInfo(
                    on_wait=keep, on_update=list(si.on_update))
            i += 1


def _build(terms=TERMS, debug_taps=False):
    nc = bass.Bass("TRN2", target_bir_lowering=False, debug=False,
                   num_devices=N_CORES)

    x_d = nc.dram_tensor("x", [B_LOC, CIN, H, W], dt.float32,
                         kind="ExternalInput")
    # weight pre-laid-out host-side as [ci, tap*256 + co]
    w_d = nc.dram_tensor("wt", [CIN, NTAP * COUT], dt.float32,
                         kind="ExternalInput")
    # s1/s2 replicated host-side to [128, tap*256+co]
    s1_d = nc.dram_tensor("s1f", [128, NTAP * COUT], dt.float32,
                          kind="ExternalInput")
    s2_d = nc.dram_tensor("s2f", [128, NTAP * COUT], dt.float32,
                          kind="ExternalInput")
    b_d = nc.dram_tensor("bias", [COUT, 1], dt.float32, kind="ExternalInput")
    y_d = nc.dram_tensor("y", [B_LOC, COUT, H, W], dt.float32,
                         kind="ExternalOutput")
    cc_in = nc.dram_tensor("cc_in", [1, 8], dt.float32)
    cc_out = nc.dram_tensor("cc_out", [1, 8], dt.float32)
    if debug_taps:
        dbg_a = nc.dram_tensor("dbg_alpha", [1, 2], dt.float32,
                               kind="ExternalOutput")
        dbg_xq = nc.dram_tensor("dbg_xq", [128, HP, WP], dt.float32,
                                kind="ExternalOutput")

    FREE_W = NTAP * COUT  # 2304
    HHALF = H // 2

    with tile.TileContext(nc, num_cores=N_CORES) as tc:
        with tc.tile_pool(name="persist", bufs=1) as pp, \
             tc.tile_pool(name="xs", bufs=3) as xsp, \
             tc.tile_pool(name="wtmp", bufs=1) as wp, \
             tc.tile_pool(name="stage", bufs=6) as sp, \
             tc.tile_pool(name="cpsum", bufs=4, space="PSUM") as cps, \
             tc.tile_pool(name="spsum", bufs=1, space="PSUM") as sps, \
             tc.tile_pool(name="dram", bufs=1, space="DRAM") as dp:

            # constants first (cheap, engines idle)
            ones_r = pp.tile([1, 128], dt.float32, tag="ones_r")
            nc.vector.memset(ones_r[:], 1.0)
            ones_c = pp.tile([128, 1], dt.float32, tag="ones_c")
            nc.vector.memset(ones_c[:], 1.0)

            # ------- pass A: stream x, abs-reduce -> AllReduce (critical) ---
            parts = pp.tile([128, B_LOC * NCIG], dt.float32, tag="parts")
            for b in range(B_LOC):
                for g in range(NCIG):
                    t = xsp.tile([128, H * W], dt.float32, tag="xs")
                    nc.sync.dma_start(t[:], x_d[b, g * 128:(g + 1) * 128, :, :])
                    nc.vector.tensor_reduce(
                        parts[:, b * NCIG + g: b * NCIG + g + 1],
                        t[:], op=Alu.add, axis=mybir.AxisListType.X,
                        apply_absolute_value=True)
            partred = pp.tile([128, 1], dt.float32, tag="partred")
            nc.vector.tensor_reduce(partred[:], parts[:], op=Alu.add,
                                    axis=mybir.AxisListType.X)
            stot_ps = sps.tile([1, 1], dt.float32, tag="stot")
            nc.tensor.matmul(stot_ps[:], partred[:], ones_c[:],
                             start=True, stop=True)
            ssb = pp.tile([1, 8], dt.float32, tag="ssb")
            nc.vector.memset(ssb[:], 0.0)
            nc.vector.tensor_copy(ssb[:, 0:1], stot_ps[:])
            nc.sync.dma_start(cc_in[:], ssb[:])
            nc.gpsimd.collective_compute(
                "AllReduce", Alu.add,
                replica_groups=[list(range(N_CORES))],
                ins=[cc_in[:]], outs=[cc_out[:]])
            sglob = pp.tile([1, 8], dt.float32, tag="sglob")
            nc.sync.dma_start(sglob[:], cc_out[:])

            # alpha = (2*mean|x|) / sqrt(127); recip = 1/alpha
            alpha_s = pp.tile([1, 1], dt.float32, tag="alpha_s")
            nc.vector.tensor_scalar(alpha_s[:], sglob[:, 0:1],
                                    float(2.0 ** -23), INV_SQRT_QP,
                                    op0=Alu.mult, op1=Alu.mult)
            recip_s = pp.tile([1, 1], dt.float32, tag="recip_s")
            nc.vector.reciprocal(recip_s[:], alpha_s[:])
            # broadcast both to [128,1] via K=1 fp32 matmuls (exact)
            al_ps = sps.tile([128, 1], dt.float32, tag="al_ps")
            nc.tensor.matmul(al_ps[:], ones_r[:], alpha_s[:],
                             start=True, stop=True)
            alpha_bc = pp.tile([128, 1], dt.float32, tag="alpha_bc")
            nc.vector.tensor_copy(alpha_bc[:], al_ps[:])
            rc_ps = sps.tile([128, 1], dt.float32, tag="rc_ps")
            nc.tensor.matmul(rc_ps[:], ones_r[:], recip_s[:],
                             start=True, stop=True)
            recip_bc = pp.tile([128, 1], dt.float32, tag="recip_bc")
            nc.vector.tensor_copy(recip_bc[:], rc_ps[:])

            # ------- xq padded buffers: zero the borders early ------------
            xq = {}
            for b in range(B_LOC):
                for g in range(NCIG):
                    t = pp.tile([128, HP, WP], dt.bfloat16, tag=f"xq{b}{g}")
                    nc.vector.memset(t[:, 0:1, :], 0.0)
                    nc.vector.memset(t[:, HP - 1:HP, :], 0.0)
                    nc.vector.memset(t[:, 1:HP - 1, 0:1], 0.0)
                    nc.vector.memset(t[:, 1:HP - 1, WP - 1:WP], 0.0)
                    xq[b, g] = t

            # ------- small input DMAs + weight prep (independent of alpha) -
            bias_t = []
            for c in range(NCOG):
                bt = pp.tile([128, 1], dt.float32, tag=f"bias{c}")
                nc.sync.dma_start(bt[:], b_d[c * 128:(c + 1) * 128, :])
                bias_t.append(bt)
            s1_t = pp.tile([128, FREE_W], dt.float32, tag="s1")
            nc.sync.dma_start(s1_t[:], s1_d[:])
            s2_t = pp.tile([128, FREE_W], dt.float32, tag="s2")
            nc.sync.dma_start(s2_t[:], s2_d[:])

            wq = {}   # (g, term) -> [128, 2304] bf16
            for g in range(NCIG):
                Wg = wp.tile([128, FREE_W], dt.float32, tag="w")
                nc.sync.dma_start(Wg[:], w_d[g * 128:(g + 1) * 128, :])
                sgnw = wp.tile([128, FREE_W], dt.float32, tag="sgnw")
                nc.vector.tensor_scalar(sgnw[:], Wg[:], 1e38, -1.0,
                                        op0=Alu.mult, op1=Alu.max)
                nc.vector.tensor_scalar(sgnw[:], sgnw[:], 1.0, None,
                                        op0=Alu.min)
                t1 = wp.tile([128, FREE_W], dt.float32, tag="t1")
                nc.vector.tensor_tensor(t1[:], sgnw[:], s1_t[:], op=Alu.mult)
                res = wp.tile([128, FREE_W], dt.float32, tag="res")
                nc.vector.tensor_tensor(res[:], Wg[:], t1[:], op=Alu.subtract)
                nc.vector.tensor_scalar(res[:], res[:], 1e38, -1.0,
                                        op0=Alu.mult, op1=Alu.max)
                nc.vector.tensor_scalar(res[:], res[:], 1.0, None,
                                        op0=Alu.min)
                nc.vector.tensor_tensor(res[:], res[:], s2_t[:], op=Alu.mult)
                bw = wp.tile([128, FREE_W], dt.float32, tag="bw")
                nc.vector.tensor_tensor(bw[:], t1[:], res[:], op=Alu.add)
                hi = pp.tile([128, FREE_W], dt.bfloat16, tag=f"hi{g}")
                nc.scalar.activation(hi[:], bw[:], Act.Copy)
                wq[g, 0] = hi
                if terms == 2:
                    lo_f = wp.tile([128, FREE_W], dt.float32, tag="sgnw")
                    nc.vector.tensor_tensor(lo_f[:], bw[:], hi[:],
                                            op=Alu.subtract)
                    lo = pp.tile([128, FREE_W], dt.bfloat16, tag=f"lo{g}")
                    nc.scalar.activation(lo[:], lo_f[:], Act.Copy)
                    wq[g, 1] = lo

            # ------- quantize: pass B re-streams x -------------------------
            for b in range(B_LOC):
                for g in range(NCIG):
                    q = xsp.tile([128, H * W], dt.float32, tag="xs")
                    nc.sync.dma_start(q[:], x_d[b, g * 128:(g + 1) * 128, :, :])
                    # q = min(x * (1/a), 127)              (DVE, in place)
                    nc.vector.tensor_scalar(q[:], q[:], recip_bc[:, 0:1],
                                            127.0, op0=Alu.mult, op1=Alu.min)
                    # q = max(q, -128) + C  (rounds RNE on the f32 write)
                    nc.vector.tensor_scalar(q[:], q[:], -128.0, ROUND_C,
                                            op0=Alu.max, op1=Alu.add)
                    # interior <- q - C as bf16 (exact small integers)
                    nc.scalar.activation(
                        xq[b, g][:, 1:HP - 1, 1:WP - 1],
                        q[:].rearrange("p (h w) -> p h w", h=H),
                        Act.Copy, bias=-ROUND_C)

            if debug_taps:
                da = pp.tile([1, 2], dt.float32, tag="dbg_a")
                nc.vector.tensor_copy(da[:, 0:1], alpha_s[:])
                nc.vector.tensor_copy(da[:, 1:2], recip_s[:])
                nc.sync.dma_start(dbg_a[:], da[:])
                dxq = pp.tile([128, HP, WP], dt.float32, tag="dbg_xq")
                nc.vector.tensor_copy(dxq[:], xq[0, 0][:])
                nc.sync.dma_start(dbg_xq[:], dxq[:])

            # ------- conv ---------------------------------------------------
            taps = [(ky, kx) for ky in range(K) for kx in range(K)]
            ep_idx = 0
            for b in range(B_LOC):
                for s in range(NSTRIP):
                    r0 = s * ROWS_PER_STRIP
                    for c in range(NCOG):
                        ps = cps.tile([128, ROWS_PER_STRIP * W], dt.float32,
                                      tag="cps")
                        n_mm = terms * NCIG * NTAP
                        i_mm = 0
                        for term in range(terms):
                            for g in range(NCIG):
                                wqt = wq[g, term]
                                for (ky, kx) in taps:
                                    tap = ky * K + kx
                                    lhsT = wqt[:, tap * COUT + c * 128:
                                               tap * COUT + c * 128 + 128]
                                    rhs = xq[b, g][:, r0 + ky:
                                                   r0 + ky + ROWS_PER_STRIP,
                                                   kx:kx + W]
                                    nc.tensor.matmul(
                                        ps[:], lhsT, rhs,
                                        start=(i_mm == 0),
                                        stop=(i_mm == n_mm - 1))
                                    i_mm += 1
                        out_t = sp.tile([128, ROWS_PER_STRIP * W], dt.float32,
                                        tag="out")
                        # y = alpha*psum + bias; alternate engines (ACT is
                        # ~2.3x slower per op but otherwise idle)
                        if ep_idx % 3 == 2:
                            nc.scalar.activation(out_t[:], ps[:], Act.Identity,
                                                 bias=bias_t[c][:, 0:1],
                                                 scale=alpha_bc[:, 0:1])
                        else:
                            nc.vector.tensor_scalar(out_t[:], ps[:],
                                                    alpha_bc[:, 0:1],
                                                    bias_t[c][:, 0:1],
                                                    op0=Alu.mult, op1=Alu.add)
                        ep_idx += 1
                        nc.sync.dma_start(
                            y_d[b, c * 128:(c + 1) * 128,
                                r0:r0 + ROWS_PER_STRIP, :],
                            out_t[:])

    _split_waits(nc, 1)
    return nc


def _prep_host_inputs(x, weight, bias, scaling_first_order,
                      scaling_second_order):
    """Layout-only host prep: shard x over batch, relayout/replicate weights."""
    w_t = np.ascontiguousarray(
        weight.transpose(1, 2, 3, 0)).reshape(CIN, NTAP * COUT)
    s1 = np.asarray(scaling_first_order, np.float32).reshape(COUT)
    s2 = np.asarray(scaling_second_order, np.float32).reshape(COUT)
    s1f = np.ascontiguousarray(
        np.broadcast_to(np.tile(s1, NTAP), (128, NTAP * COUT)))
    s2f = np.ascontiguousarray(
        np.broadcast_to(np.tile(s2, NTAP), (128, NTAP * COUT)))
    b_r = np.asarray(bias, np.float32).reshape(COUT, 1)
    x = np.asarray(x, np.float32)
    in_maps = []
    for i in range(N_CORES):
        in_maps.append({
            "x": np.ascontiguousarray(x[i * B_LOC:(i + 1) * B_LOC]),
            "wt": w_t, "s1f": s1f, "s2f": s2f, "bias": b_r,
        })
    return in_maps


def _install_ntff_hook():
    import types
    try:
        import antenv.axon_hooks  # noqa: F401
        return
    except ImportError:
        pass
    from trn_agent_boot.trn_boot import _ntff_profile_via_ctypes
    hook = _ntff_profile_via_ctypes('/opt/axon/libaxon_pjrt.so')
    mod = types.ModuleType('antenv.axon_hooks')
    mod.get_axon_ntff_profile_hook = lambda: hook
    mod.set_axon_ntff_profile_hook = lambda h: None
    sys.modules['antenv.axon_hooks'] = mod


def _run(in_maps, terms=TERMS, trace=False, debug_taps=False):
    from concourse.bass_utils import run_bass_kernel_spmd
    if trace:
        _install_ntff_hook()
        from concourse import bass_utils
        bass_utils.upload_artifacts = lambda tmpdir: f"local:{tmpdir}"
    nc = _build(terms, debug_taps=debug_taps)
    return run_bass_kernel_spmd(nc, in_maps, list(range(N_CORES)),
                                trace=trace)


def kernel(x, weight, bias, scaling_first_order, scaling_second_order):
    in_maps = _prep_host_inputs(x, weight, bias, scaling_first_order,
                                scaling_second_order)
    res = _run(in_maps, TERMS, trace=False)
    return np.concatenate([res.results[i]["y"] for i in range(N_CORES)],
                          axis=0)


# revision 17
# speedup vs baseline: 1.7546x; 1.0317x over previous
"""Trainium2 Bass kernel for nn_BNNConv2d (LSQ 8-bit act quant + 2nd-order
binary-residual weight quant + 3x3 conv, NCHW, pad 1, stride 1) on 8 cores.

Strategy
--------
Data-parallel over batch: 16 images -> 2 per NeuronCore; weights/scalings
replicated.  Per core:

  1. x shard (2,256,64,64) f32 streams to SBUF; |x| partial sums reduce on
     DVE/GpSimd; cross-partition sum via an fp32 ones-matmul; the 8 per-core
     scalars AllReduce to the global sum -> alpha = 2*mean|x|/sqrt(127).
  2. Activations quantize to INTEGER-VALUED bf16 (exact: integers in
     [-128,127]): q = ((x*(1/a) clamped) + 1.5*2^23) - 1.5*2^23 (RNE round
     trick), written into zero-padded [128, 66, 66] images per (image,
     ci-half).  Pipelined DVE -> GpSimd -> ScalarE in row-halves.
  3. Effective weight bw = sign(w)*s1 + sign(w - sign(w)*s1)*s2 built on DVE
     (exact sign via clamp(w*1e38, -1, 1)), cast to bf16 (terms=1), or split
     bw = hi + lo into two bf16 terms (terms=2, near-exact).  bf16 x bf16
     products against integer activations are exact; PSUM accumulates fp32.
     The dominant error vs the jax reference is the fp32 summation-order
     difference in alpha (XLA CPU mean), which flips ~1e-4 of the 16.7M
     quantization roundings -> ~3e-3 absmax rel err for either terms setting.
  4. Conv = 9 shifted matmuls per ci-half per term accumulated in PSUM over
     strips of 8 output rows (N=512 = one PSUM bank), 2 Cout tiles of 128.
  5. Epilogue y = alpha*psum + bias on DVE/ScalarE (alternating), DMA out.

The walrus build in this container only supports ONE sync wait per
instruction; _split_waits() hoists extra waits onto same-engine NOPs.
"""

import sys

for _p in ("/opt/trn_rl_repo", "/root/.axon_site"):
    if _p not in sys.path:
        sys.path.insert(0, _p)

import numpy as np

import concourse.bass as bass
import concourse.mybir as mybir
import concourse.tile as tile
from concourse.bass import _add_dep_helper

dt = mybir.dt
Alu = mybir.AluOpType
Act = mybir.ActivationFunctionType

N_CORES = 8
B, CIN, H, W = 16, 256, 64, 64
COUT, K = 256, 3
B_LOC = B // N_CORES            # images per core
NCIG = CIN // 128               # ci groups of 128 partitions
NCOG = COUT // 128              # co tiles of 128
HP, WP = H + 2, W + 2           # padded image
ROWS_PER_STRIP = 8              # 8 rows x 64 cols = 512 = one PSUM bank
NSTRIP = H // ROWS_PER_STRIP
NTAP = K * K
TERMS = 1                       # bf16 weight terms (1: fast, 2: hi+lo split)

# round-to-int magic: v + 1.5*2^23 stays in [2^23, 2^24) for v in [-128,127],
# where the fp32 ulp is exactly 1.0 -> the add rounds v to the nearest int (RNE)
ROUND_C = float(2 ** 23 + 2 ** 22)
INV_SQRT_QP = float(np.float32(1.0) / np.float32(np.sqrt(127.0)))


def _split_waits(nc, max_waits=1):
    """Split >max_waits sync waits per instruction onto same-engine NOPs."""
    for bb in nc.main_func.blocks:
        il = bb.instructions
        i = 0
        while i < len(il):
            ins = il[i]
            si = ins.sync_info
            if si is not None and si.on_wait and len(si.on_wait) > max_waits:
                waits = list(si.on_wait)
                extra, keep = waits[:-max_waits], waits[-max_waits:]
                k = 0
                while extra:
                    chunk, extra = extra[:max_waits], extra[max_waits:]
                    nop = mybir.InstNoOp(
                        name=f"{ins.name}-wsplit{k}", ins=[], outs=[])
                    nop.engine = ins.engine
                    nop.sync_info = mybir.SyncInfo(on_wait=chunk, on_update=[])
                    il.insert(i, nop)
                    i += 1
                    k += 1
                ins.sync_info = mybir.SyncInfo(
                    on_wait=keep, on_update=list(si.on_update))
            i += 1


def _build(terms=TERMS, debug_taps=False):
    nc = bass.Bass("TRN2", target_bir_lowering=False, debug=False,
                   num_devices=N_CORES)

    x_d = nc.dram_tensor("x", [B_LOC, CIN, H, W], dt.float32,
                         kind="ExternalInput")
    # weight pre-laid-out host-side as [ci, tap*256 + co]
    w_d = nc.dram_tensor("wt", [CIN, NTAP * COUT], dt.float32,
                         kind="ExternalInput")
    # s1/s2 replicated host-side to [128, tap*256+co]
    s1_d = nc.dram_tensor("s1f", [128, NTAP * COUT], dt.float32,
                          kind="ExternalInput")
    s2_d = nc.dram_tensor("s2f", [128, NTAP * COUT], dt.float32,
                          kind="ExternalInput")
    b_d = nc.dram_tensor("bias", [COUT, 1], dt.float32, kind="ExternalInput")
    y_d = nc.dram_tensor("y", [B_LOC, COUT, H, W], dt.float32,
                         kind="ExternalOutput")
    cc_in = nc.dram_tensor("cc_in", [1, 8], dt.float32)
    cc_out = nc.dram_tensor("cc_out", [1, 8], dt.float32)
    if debug_taps:
        dbg_a = nc.dram_tensor("dbg_alpha", [1, 2], dt.float32,
                               kind="ExternalOutput")
        dbg_xq = nc.dram_tensor("dbg_xq", [128, HP, WP], dt.float32,
                                kind="ExternalOutput")

    FREE_W = NTAP * COUT  # 2304
    HHALF = H // 2

    with tile.TileContext(nc, num_cores=N_CORES) as tc:
        with tc.tile_pool(name="persist", bufs=1) as pp, \
             tc.tile_pool(name="xs", bufs=3) as xsp, \
             tc.tile_pool(name="wtmp", bufs=1) as wp, \
             tc.tile_pool(name="stage", bufs=6) as sp, \
             tc.tile_pool(name="cpsum", bufs=8, space="PSUM") as cps, \
             tc.tile_pool(name="dram", bufs=1, space="DRAM") as dp:

            # constants first (cheap, engines idle)
            ones_r = pp.tile([1, 128], dt.float32, tag="ones_r")
            nc.vector.memset(ones_r[:], 1.0)
            ones_c = pp.tile([128, 1], dt.float32, tag="ones_c")
            nc.vector.memset(ones_c[:], 1.0)

            # ------- pass A: stream x, abs-reduce -> AllReduce (critical) ---
            parts = pp.tile([128, B_LOC * NCIG], dt.float32, tag="parts")
            for b in range(B_LOC):
                for g in range(NCIG):
                    t = xsp.tile([128, H * W], dt.float32, tag="xs")
                    nc.sync.dma_start(t[:], x_d[b, g * 128:(g + 1) * 128, :, :])
                    nc.vector.tensor_reduce(
                        parts[:, b * NCIG + g: b * NCIG + g + 1],
                        t[:], op=Alu.add, axis=mybir.AxisListType.X,
                        apply_absolute_value=True)
            partred = pp.tile([128, 1], dt.float32, tag="partred")
            nc.vector.tensor_reduce(partred[:], parts[:], op=Alu.add,
                                    axis=mybir.AxisListType.X)
            stot_ps = cps.tile([1, 1], dt.float32, tag="cps")
            nc.tensor.matmul(stot_ps[:], partred[:], ones_c[:],
                             start=True, stop=True)
            ssb = pp.tile([1, 8], dt.float32, tag="ssb")
            nc.vector.memset(ssb[:], 0.0)
            nc.vector.tensor_copy(ssb[:, 0:1], stot_ps[:])
            nc.sync.dma_start(cc_in[:], ssb[:])
            cc_inst = nc.gpsimd.collective_compute(
                "AllReduce", Alu.add,
                replica_groups=[list(range(N_CORES))],
                ins=[cc_in[:]], outs=[cc_out[:]])
            sglob = pp.tile([1, 8], dt.float32, tag="sglob")
            nc.sync.dma_start(sglob[:], cc_out[:])

            # alpha = (2*mean|x|) / sqrt(127); recip = 1/alpha
            alpha_s = pp.tile([1, 1], dt.float32, tag="alpha_s")
            nc.vector.tensor_scalar(alpha_s[:], sglob[:, 0:1],
                                    float(2.0 ** -23), INV_SQRT_QP,
                                    op0=Alu.mult, op1=Alu.mult)
            recip_s = pp.tile([1, 1], dt.float32, tag="recip_s")
            nc.vector.reciprocal(recip_s[:], alpha_s[:])
            # broadcast both to [128,1] via K=1 fp32 matmuls (exact)
            al_ps = cps.tile([128, 1], dt.float32, tag="cps")
            nc.tensor.matmul(al_ps[:], ones_r[:], alpha_s[:],
                             start=True, stop=True)
            alpha_bc = pp.tile([128, 1], dt.float32, tag="alpha_bc")
            nc.vector.tensor_copy(alpha_bc[:], al_ps[:])
            rc_ps = cps.tile([128, 1], dt.float32, tag="cps")
            nc.tensor.matmul(rc_ps[:], ones_r[:], recip_s[:],
                             start=True, stop=True)
            recip_bc = pp.tile([128, 1], dt.float32, tag="recip_bc")
            nc.vector.tensor_copy(recip_bc[:], rc_ps[:])

            # ------- xq padded buffers: zero the borders early ------------
            xq = {}
            for b in range(B_LOC):
                for g in range(NCIG):
                    t = pp.tile([128, HP, WP], dt.bfloat16, tag=f"xq{b}{g}")
                    nc.vector.memset(t[:, 0:1, :], 0.0)
                    nc.vector.memset(t[:, HP - 1:HP, :], 0.0)
                    nc.vector.memset(t[:, 1:HP - 1, 0:1], 0.0)
                    nc.vector.memset(t[:, 1:HP - 1, WP - 1:WP], 0.0)
                    xq[b, g] = t

            # ------- small input DMAs + weight prep (independent of alpha) -
            bias_t = []
            for c in range(NCOG):
                bt = pp.tile([128, 1], dt.float32, tag=f"bias{c}")
                nc.sync.dma_start(bt[:], b_d[c * 128:(c + 1) * 128, :])
                bias_t.append(bt)
            s1_t = pp.tile([128, FREE_W], dt.float32, tag="s1")
            nc.sync.dma_start(s1_t[:], s1_d[:])
            s2_t = pp.tile([128, FREE_W], dt.float32, tag="s2")
            nc.sync.dma_start(s2_t[:], s2_d[:])

            wq = {}   # (g, term) -> [128, 2304] bf16
            for g in range(NCIG):
                Wg = wp.tile([128, FREE_W], dt.float32, tag="w")
                nc.sync.dma_start(Wg[:], w_d[g * 128:(g + 1) * 128, :])
                sgnw = wp.tile([128, FREE_W], dt.float32, tag="sgnw")
                nc.vector.tensor_scalar(sgnw[:], Wg[:], 1e38, -1.0,
                                        op0=Alu.mult, op1=Alu.max)
                nc.vector.tensor_scalar(sgnw[:], sgnw[:], 1.0, None,
                                        op0=Alu.min)
                t1 = wp.tile([128, FREE_W], dt.float32, tag="t1")
                nc.vector.tensor_tensor(t1[:], sgnw[:], s1_t[:], op=Alu.mult)
                res = wp.tile([128, FREE_W], dt.float32, tag="res")
                nc.vector.tensor_tensor(res[:], Wg[:], t1[:], op=Alu.subtract)
                nc.vector.tensor_scalar(res[:], res[:], 1e38, -1.0,
                                        op0=Alu.mult, op1=Alu.max)
                nc.vector.tensor_scalar(res[:], res[:], 1.0, None,
                                        op0=Alu.min)
                nc.vector.tensor_tensor(res[:], res[:], s2_t[:], op=Alu.mult)
                bw = wp.tile([128, FREE_W], dt.float32, tag="bw")
                nc.vector.tensor_tensor(bw[:], t1[:], res[:], op=Alu.add)
                hi = pp.tile([128, FREE_W], dt.bfloat16, tag=f"hi{g}")
                nc.scalar.activation(hi[:], bw[:], Act.Copy)
                wq[g, 0] = hi
                if terms == 2:
                    lo_f = wp.tile([128, FREE_W], dt.float32, tag="sgnw")
                    nc.vector.tensor_tensor(lo_f[:], bw[:], hi[:],
                                            op=Alu.subtract)
                    lo = pp.tile([128, FREE_W], dt.bfloat16, tag=f"lo{g}")
                    nc.scalar.activation(lo[:], lo_f[:], Act.Copy)
                    wq[g, 1] = lo

            # ------- quantize: pass B re-streams x -------------------------
            for b in range(B_LOC):
                for g in range(NCIG):
                    q = xsp.tile([128, H * W], dt.float32, tag="xs")
                    dma_b = nc.sync.dma_start(
                        q[:], x_d[b, g * 128:(g + 1) * 128, :, :])
                    _add_dep_helper(dma_b.ins, cc_inst.ins, sync=True,
                                    reason="keep SDMA clear during AllReduce")
                    # q = min(x * (1/a), 127)              (DVE, in place)
                    nc.vector.tensor_scalar(q[:], q[:], recip_bc[:, 0:1],
                                            127.0, op0=Alu.mult, op1=Alu.min)
                    # q = max(q, -128) + C  (rounds RNE on the f32 write)
                    nc.vector.tensor_scalar(q[:], q[:], -128.0, ROUND_C,
                                            op0=Alu.max, op1=Alu.add)
                    # interior <- q - C as bf16 (exact small integers)
                    nc.scalar.activation(
                        xq[b, g][:, 1:HP - 1, 1:WP - 1],
                        q[:].rearrange("p (h w) -> p h w", h=H),
                        Act.Copy, bias=-ROUND_C)

            if debug_taps:
                da = pp.tile([1, 2], dt.float32, tag="dbg_a")
                nc.vector.tensor_copy(da[:, 0:1], alpha_s[:])
                nc.vector.tensor_copy(da[:, 1:2], recip_s[:])
                nc.sync.dma_start(dbg_a[:], da[:])
                dxq = pp.tile([128, HP, WP], dt.float32, tag="dbg_xq")
                nc.vector.tensor_copy(dxq[:], xq[0, 0][:])
                nc.sync.dma_start(dbg_xq[:], dxq[:])

            # ------- conv: blocks of 2 strips share each weight load -------
            taps = [(ky, kx) for ky in range(K) for kx in range(K)]
            ep_idx = 0
            n_mm = terms * NCIG * NTAP
            for b in range(B_LOC):
                for blk in range(NSTRIP // 2):
                    strips = (2 * blk, 2 * blk + 1)
                    for c in range(NCOG):
                        pss = []
                        for _si in strips:
                            ps_t = cps.tile([128, ROWS_PER_STRIP * W],
                                            dt.float32, tag="cps")
                            pss.append(ps_t)
                        i_mm = 0
                        for term in range(terms):
                            for g in range(NCIG):
                                wqt = wq[g, term]
                                for (ky, kx) in taps:
                                    tap = ky * K + kx
                                    lhsT = wqt[:, tap * COUT + c * 128:
                                               tap * COUT + c * 128 + 128]
                                    for si, s in enumerate(strips):
                                        r0 = s * ROWS_PER_STRIP
                                        rhs = xq[b, g][:, r0 + ky:
                                                       r0 + ky +
                                                       ROWS_PER_STRIP,
                                                       kx:kx + W]
                                        nc.tensor.matmul(
                                            pss[si][:], lhsT, rhs,
                                            start=(i_mm == 0),
                                            stop=(i_mm == n_mm - 1),
                                            skip_group_check=True)
                                    i_mm += 1
                        for si, s in enumerate(strips):
                            r0 = s * ROWS_PER_STRIP
                            out_t = sp.tile([128, ROWS_PER_STRIP * W],
                                            dt.float32, tag="out")
                            # y = alpha*psum + bias; alternate engines (ACT
                            # is ~2.3x slower per op but otherwise idle)
                            if ep_idx % 3 == 2:
                                nc.scalar.activation(out_t[:], pss[si][:],
                                                     Act.Identity,
                                                     bias=bias_t[c][:, 0:1],
                                                     scale=alpha_bc[:, 0:1])
                            else:
                                nc.vector.tensor_scalar(out_t[:], pss[si][:],
                                                        alpha_bc[:, 0:1],
                                                        bias_t[c][:, 0:1],
                                                        op0=Alu.mult,
                                                        op1=Alu.add)
                            ep_idx += 1
                            nc.sync.dma_start(
                                y_d[b, c * 128:(c + 1) * 128,
                                    r0:r0 + ROWS_PER_STRIP, :],
                                out_t[:])

    _split_waits(nc, 1)
    return nc


def _prep_host_inputs(x, weight, bias, scaling_first_order,
                      scaling_second_order):
    """Layout-only host prep: shard x over batch, relayout/replicate weights."""
    w_t = np.ascontiguousarray(
        weight.transpose(1, 2, 3, 0)).reshape(CIN, NTAP * COUT)
    s1 = np.asarray(scaling_first_order, np.float32).reshape(COUT)
    s2 = np.asarray(scaling_second_order, np.float32).reshape(COUT)
    s1f = np.ascontiguousarray(
        np.broadcast_to(np.tile(s1, NTAP), (128, NTAP * COUT)))
    s2f = np.ascontiguousarray(
        np.broadcast_to(np.tile(s2, NTAP), (128, NTAP * COUT)))
    b_r = np.asarray(bias, np.float32).reshape(COUT, 1)
    x = np.asarray(x, np.float32)
    in_maps = []
    for i in range(N_CORES):
        in_maps.append({
            "x": np.ascontiguousarray(x[i * B_LOC:(i + 1) * B_LOC]),
            "wt": w_t, "s1f": s1f, "s2f": s2f, "bias": b_r,
        })
    return in_maps


def _install_ntff_hook():
    import types
    try:
        import antenv.axon_hooks  # noqa: F401
        return
    except ImportError:
        pass
    from trn_agent_boot.trn_boot import _ntff_profile_via_ctypes
    hook = _ntff_profile_via_ctypes('/opt/axon/libaxon_pjrt.so')
    mod = types.ModuleType('antenv.axon_hooks')
    mod.get_axon_ntff_profile_hook = lambda: hook
    mod.set_axon_ntff_profile_hook = lambda h: None
    sys.modules['antenv.axon_hooks'] = mod


def _run(in_maps, terms=TERMS, trace=False, debug_taps=False):
    from concourse.bass_utils import run_bass_kernel_spmd
    if trace:
        _install_ntff_hook()
        from concourse import bass_utils
        bass_utils.upload_artifacts = lambda tmpdir: f"local:{tmpdir}"
    nc = _build(terms, debug_taps=debug_taps)
    return run_bass_kernel_spmd(nc, in_maps, list(range(N_CORES)),
                                trace=trace)


def kernel(x, weight, bias, scaling_first_order, scaling_second_order):
    in_maps = _prep_host_inputs(x, weight, bias, scaling_first_order,
                                scaling_second_order)
    res = _run(in_maps, TERMS, trace=False)
    return np.concatenate([res.results[i]["y"] for i in range(N_CORES)],
                          axis=0)


# revision 18
# speedup vs baseline: 1.7765x; 1.0124x over previous
"""Trainium2 Bass kernel for nn_BNNConv2d (LSQ 8-bit act quant + 2nd-order
binary-residual weight quant + 3x3 conv, NCHW, pad 1, stride 1) on 8 cores.

Strategy
--------
Data-parallel over batch: 16 images -> 2 per NeuronCore; weights/scalings
replicated.  Per core:

  1. x shard (2,256,64,64) f32 streams to SBUF; |x| partial sums reduce on
     DVE/GpSimd; cross-partition sum via an fp32 ones-matmul; the 8 per-core
     scalars AllReduce to the global sum -> alpha = 2*mean|x|/sqrt(127).
  2. Activations quantize to INTEGER-VALUED bf16 (exact: integers in
     [-128,127]): q = ((x*(1/a) clamped) + 1.5*2^23) - 1.5*2^23 (RNE round
     trick), written into zero-padded [128, 66, 66] images per (image,
     ci-half).  Pipelined DVE -> GpSimd -> ScalarE in row-halves.
  3. Effective weight bw = sign(w)*s1 + sign(w - sign(w)*s1)*s2 built on DVE
     (exact sign via clamp(w*1e38, -1, 1)), cast to bf16 (terms=1), or split
     bw = hi + lo into two bf16 terms (terms=2, near-exact).  bf16 x bf16
     products against integer activations are exact; PSUM accumulates fp32.
     The dominant error vs the jax reference is the fp32 summation-order
     difference in alpha (XLA CPU mean), which flips ~1e-4 of the 16.7M
     quantization roundings -> ~3e-3 absmax rel err for either terms setting.
  4. Conv = 9 shifted matmuls per ci-half per term accumulated in PSUM over
     strips of 8 output rows (N=512 = one PSUM bank), 2 Cout tiles of 128.
  5. Epilogue y = alpha*psum + bias on DVE/ScalarE (alternating), DMA out.

The walrus build in this container only supports ONE sync wait per
instruction; _split_waits() hoists extra waits onto same-engine NOPs.
"""

import sys

for _p in ("/opt/trn_rl_repo", "/root/.axon_site"):
    if _p not in sys.path:
        sys.path.insert(0, _p)

import numpy as np

import concourse.bass as bass
import concourse.mybir as mybir
import concourse.tile as tile
from concourse.bass import _add_dep_helper

dt = mybir.dt
Alu = mybir.AluOpType
Act = mybir.ActivationFunctionType

N_CORES = 8
B, CIN, H, W = 16, 256, 64, 64
COUT, K = 256, 3
B_LOC = B // N_CORES            # images per core
NCIG = CIN // 128               # ci groups of 128 partitions
NCOG = COUT // 128              # co tiles of 128
HP, WP = H + 2, W + 2           # padded image
ROWS_PER_STRIP = 8              # 8 rows x 64 cols = 512 = one PSUM bank
NSTRIP = H // ROWS_PER_STRIP
NTAP = K * K
TERMS = 1                       # bf16 weight terms (1: fast, 2: hi+lo split)

# round-to-int magic: v + 1.5*2^23 stays in [2^23, 2^24) for v in [-128,127],
# where the fp32 ulp is exactly 1.0 -> the add rounds v to the nearest int (RNE)
ROUND_C = float(2 ** 23 + 2 ** 22)
INV_SQRT_QP = float(np.float32(1.0) / np.float32(np.sqrt(127.0)))


def _split_waits(nc, max_waits=1):
    """Split >max_waits sync waits per instruction onto same-engine NOPs."""
    for bb in nc.main_func.blocks:
        il = bb.instructions
        i = 0
        while i < len(il):
            ins = il[i]
            si = ins.sync_info
            if si is not None and si.on_wait and len(si.on_wait) > max_waits:
                waits = list(si.on_wait)
                extra, keep = waits[:-max_waits], waits[-max_waits:]
                k = 0
                while extra:
                    chunk, extra = extra[:max_waits], extra[max_waits:]
                    nop = mybir.InstNoOp(
                        name=f"{ins.name}-wsplit{k}", ins=[], outs=[])
                    nop.engine = ins.engine
                    nop.sync_info = mybir.SyncInfo(on_wait=chunk, on_update=[])
                    il.insert(i, nop)
                    i += 1
                    k += 1
                ins.sync_info = mybir.SyncInfo(
                    on_wait=keep, on_update=list(si.on_update))
            i += 1


def _build(terms=TERMS, debug_taps=False):
    nc = bass.Bass("TRN2", target_bir_lowering=False, debug=False,
                   num_devices=N_CORES)

    x_d = nc.dram_tensor("x", [B_LOC, CIN, H, W], dt.float32,
                         kind="ExternalInput")
    # weight pre-laid-out host-side as [ci, tap*256 + co]
    w_d = nc.dram_tensor("wt", [CIN, NTAP * COUT], dt.float32,
                         kind="ExternalInput")
    # s1/s2 replicated host-side to [128, tap*256+co]
    s1_d = nc.dram_tensor("s1f", [128, NTAP * COUT], dt.float32,
                          kind="ExternalInput")
    s2_d = nc.dram_tensor("s2f", [128, NTAP * COUT], dt.float32,
                          kind="ExternalInput")
    b_d = nc.dram_tensor("bias", [COUT, 1], dt.float32, kind="ExternalInput")
    y_d = nc.dram_tensor("y", [B_LOC, COUT, H, W], dt.float32,
                         kind="ExternalOutput")
    cc_in = nc.dram_tensor("cc_in", [1, 8], dt.float32)
    cc_out = nc.dram_tensor("cc_out", [1, 8], dt.float32,
                            addr_space="Shared")
    if debug_taps:
        dbg_a = nc.dram_tensor("dbg_alpha", [1, 2], dt.float32,
                               kind="ExternalOutput")
        dbg_xq = nc.dram_tensor("dbg_xq", [128, HP, WP], dt.float32,
                                kind="ExternalOutput")

    FREE_W = NTAP * COUT  # 2304
    HHALF = H // 2

    with tile.TileContext(nc, num_cores=N_CORES) as tc:
        with tc.tile_pool(name="persist", bufs=1) as pp, \
             tc.tile_pool(name="xs", bufs=3) as xsp, \
             tc.tile_pool(name="wtmp", bufs=1) as wp, \
             tc.tile_pool(name="stage", bufs=8) as sp, \
             tc.tile_pool(name="cpsum", bufs=8, space="PSUM") as cps, \
             tc.tile_pool(name="dram", bufs=1, space="DRAM") as dp:

            # constants first (cheap, engines idle)
            ones_r = pp.tile([1, 128], dt.float32, tag="ones_r")
            nc.vector.memset(ones_r[:], 1.0)
            ones_c = pp.tile([128, 1], dt.float32, tag="ones_c")
            nc.vector.memset(ones_c[:], 1.0)

            # ------- pass A: stream x, abs-reduce -> AllReduce (critical) ---
            parts = pp.tile([128, B_LOC * NCIG], dt.float32, tag="parts")
            for b in range(B_LOC):
                for g in range(NCIG):
                    t = xsp.tile([128, H * W], dt.float32, tag="xs")
                    nc.sync.dma_start(t[:], x_d[b, g * 128:(g + 1) * 128, :, :])
                    nc.vector.tensor_reduce(
                        parts[:, b * NCIG + g: b * NCIG + g + 1],
                        t[:], op=Alu.add, axis=mybir.AxisListType.X,
                        apply_absolute_value=True)
            partred = pp.tile([128, 1], dt.float32, tag="partred")
            nc.vector.tensor_reduce(partred[:], parts[:], op=Alu.add,
                                    axis=mybir.AxisListType.X)
            stot_ps = cps.tile([1, 1], dt.float32, tag="cps")
            nc.tensor.matmul(stot_ps[:], partred[:], ones_c[:],
                             start=True, stop=True)
            ssb = pp.tile([1, 8], dt.float32, tag="ssb")
            nc.vector.memset(ssb[:], 0.0)
            nc.vector.tensor_copy(ssb[:, 0:1], stot_ps[:])
            nc.sync.dma_start(cc_in[:], ssb[:])
            cc_inst = nc.gpsimd.collective_compute(
                "AllReduce", Alu.add,
                replica_groups=[list(range(N_CORES))],
                ins=[cc_in[:]], outs=[cc_out[:]])
            sglob = pp.tile([1, 8], dt.float32, tag="sglob")
            nc.sync.dma_start(sglob[:], cc_out[:])

            # alpha = (2*mean|x|) / sqrt(127); recip = 1/alpha
            alpha_s = pp.tile([1, 1], dt.float32, tag="alpha_s")
            nc.vector.tensor_scalar(alpha_s[:], sglob[:, 0:1],
                                    float(2.0 ** -23), INV_SQRT_QP,
                                    op0=Alu.mult, op1=Alu.mult)
            recip_s = pp.tile([1, 1], dt.float32, tag="recip_s")
            nc.vector.reciprocal(recip_s[:], alpha_s[:])
            # broadcast both to [128,1] via K=1 fp32 matmuls (exact)
            al_ps = cps.tile([128, 1], dt.float32, tag="cps")
            nc.tensor.matmul(al_ps[:], ones_r[:], alpha_s[:],
                             start=True, stop=True)
            alpha_bc = pp.tile([128, 1], dt.float32, tag="alpha_bc")
            nc.vector.tensor_copy(alpha_bc[:], al_ps[:])
            rc_ps = cps.tile([128, 1], dt.float32, tag="cps")
            nc.tensor.matmul(rc_ps[:], ones_r[:], recip_s[:],
                             start=True, stop=True)
            recip_bc = pp.tile([128, 1], dt.float32, tag="recip_bc")
            nc.vector.tensor_copy(recip_bc[:], rc_ps[:])

            # ------- xq padded buffers: zero the borders early ------------
            xq = {}
            for b in range(B_LOC):
                for g in range(NCIG):
                    t = pp.tile([128, HP, WP], dt.bfloat16, tag=f"xq{b}{g}")
                    nc.vector.memset(t[:, 0:1, :], 0.0)
                    nc.vector.memset(t[:, HP - 1:HP, :], 0.0)
                    nc.vector.memset(t[:, 1:HP - 1, 0:1], 0.0)
                    nc.vector.memset(t[:, 1:HP - 1, WP - 1:WP], 0.0)
                    xq[b, g] = t

            # ------- small input DMAs + weight prep (independent of alpha) -
            bias_t = []
            for c in range(NCOG):
                bt = pp.tile([128, 1], dt.float32, tag=f"bias{c}")
                nc.sync.dma_start(bt[:], b_d[c * 128:(c + 1) * 128, :])
                bias_t.append(bt)
            s1_t = pp.tile([128, FREE_W], dt.float32, tag="s1")
            nc.sync.dma_start(s1_t[:], s1_d[:])
            s2_t = pp.tile([128, FREE_W], dt.float32, tag="s2")
            nc.sync.dma_start(s2_t[:], s2_d[:])

            wq = {}   # (g, term) -> [128, 2304] bf16
            for g in range(NCIG):
                Wg = wp.tile([128, FREE_W], dt.float32, tag="w")
                nc.sync.dma_start(Wg[:], w_d[g * 128:(g + 1) * 128, :])
                sgnw = wp.tile([128, FREE_W], dt.float32, tag="sgnw")
                nc.vector.tensor_scalar(sgnw[:], Wg[:], 1e38, -1.0,
                                        op0=Alu.mult, op1=Alu.max)
                nc.vector.tensor_scalar(sgnw[:], sgnw[:], 1.0, None,
                                        op0=Alu.min)
                t1 = wp.tile([128, FREE_W], dt.float32, tag="t1")
                nc.vector.tensor_tensor(t1[:], sgnw[:], s1_t[:], op=Alu.mult)
                res = wp.tile([128, FREE_W], dt.float32, tag="res")
                nc.vector.tensor_tensor(res[:], Wg[:], t1[:], op=Alu.subtract)
                nc.vector.tensor_scalar(res[:], res[:], 1e38, -1.0,
                                        op0=Alu.mult, op1=Alu.max)
                nc.vector.tensor_scalar(res[:], res[:], 1.0, None,
                                        op0=Alu.min)
                nc.vector.tensor_tensor(res[:], res[:], s2_t[:], op=Alu.mult)
                bw = wp.tile([128, FREE_W], dt.float32, tag="bw")
                nc.vector.tensor_tensor(bw[:], t1[:], res[:], op=Alu.add)
                hi = pp.tile([128, FREE_W], dt.bfloat16, tag=f"hi{g}")
                nc.scalar.activation(hi[:], bw[:], Act.Copy)
                wq[g, 0] = hi
                if terms == 2:
                    lo_f = wp.tile([128, FREE_W], dt.float32, tag="sgnw")
                    nc.vector.tensor_tensor(lo_f[:], bw[:], hi[:],
                                            op=Alu.subtract)
                    lo = pp.tile([128, FREE_W], dt.bfloat16, tag=f"lo{g}")
                    nc.scalar.activation(lo[:], lo_f[:], Act.Copy)
                    wq[g, 1] = lo

            # ------- quantize: pass B re-streams x -------------------------
            for b in range(B_LOC):
                for g in range(NCIG):
                    q = xsp.tile([128, H * W], dt.float32, tag="xs")
                    dma_b = nc.sync.dma_start(
                        q[:], x_d[b, g * 128:(g + 1) * 128, :, :])
                    _add_dep_helper(dma_b.ins, cc_inst.ins, sync=True,
                                    reason="keep SDMA clear during AllReduce")
                    # image 0 quantizes in row-halves so the first conv
                    # strips can start as soon as alpha lands; image 1 in
                    # full tiles (fewer instructions, fully overlapped).
                    halves = ((0, HHALF), (HHALF, HHALF)) if b == 0 \
                        else ((0, H),)
                    for h0, hn in halves:
                        sl = q[:, h0 * W:(h0 + hn) * W]
                        # q = min(x * (1/a), 127)          (DVE, in place)
                        nc.vector.tensor_scalar(sl, sl, recip_bc[:, 0:1],
                                                127.0, op0=Alu.mult,
                                                op1=Alu.min)
                        # q = max(q, -128) + C (rounds RNE on the f32 write)
                        nc.vector.tensor_scalar(sl, sl, -128.0, ROUND_C,
                                                op0=Alu.max, op1=Alu.add)
                        # interior <- q - C as bf16 (exact small integers)
                        nc.scalar.activation(
                            xq[b, g][:, 1 + h0:1 + h0 + hn, 1:WP - 1],
                            sl.rearrange("p (h w) -> p h w", h=hn),
                            Act.Copy, bias=-ROUND_C)

            if debug_taps:
                da = pp.tile([1, 2], dt.float32, tag="dbg_a")
                nc.vector.tensor_copy(da[:, 0:1], alpha_s[:])
                nc.vector.tensor_copy(da[:, 1:2], recip_s[:])
                nc.sync.dma_start(dbg_a[:], da[:])
                dxq = pp.tile([128, HP, WP], dt.float32, tag="dbg_xq")
                nc.vector.tensor_copy(dxq[:], xq[0, 0][:])
                nc.sync.dma_start(dbg_xq[:], dxq[:])

            # ------- conv: blocks of 2 strips share each weight load -------
            taps = [(ky, kx) for ky in range(K) for kx in range(K)]
            ep_idx = 0
            n_mm = terms * NCIG * NTAP
            for b in range(B_LOC):
                for blk in range(NSTRIP // 2):
                    strips = (2 * blk, 2 * blk + 1)
                    for c in range(NCOG):
                        pss = []
                        for _si in strips:
                            ps_t = cps.tile([128, ROWS_PER_STRIP * W],
                                            dt.float32, tag="cps")
                            pss.append(ps_t)
                        i_mm = 0
                        for term in range(terms):
                            for g in range(NCIG):
                                wqt = wq[g, term]
                                for (ky, kx) in taps:
                                    tap = ky * K + kx
                                    lhsT = wqt[:, tap * COUT + c * 128:
                                               tap * COUT + c * 128 + 128]
                                    for si, s in enumerate(strips):
                                        r0 = s * ROWS_PER_STRIP
                                        rhs = xq[b, g][:, r0 + ky:
                                                       r0 + ky +
                                                       ROWS_PER_STRIP,
                                                       kx:kx + W]
                                        nc.tensor.matmul(
                                            pss[si][:], lhsT, rhs,
                                            start=(i_mm == 0),
                                            stop=(i_mm == n_mm - 1),
                                            skip_group_check=True)
                                    i_mm += 1
                        for si, s in enumerate(strips):
                            r0 = s * ROWS_PER_STRIP
                            out_t = sp.tile([128, ROWS_PER_STRIP * W],
                                            dt.float32, tag="out")
                            # y = alpha*psum + bias; alternate engines (ACT
                            # is ~2.3x slower per op but otherwise idle)
                            if ep_idx % 3 == 2:
                                nc.scalar.activation(out_t[:], pss[si][:],
                                                     Act.Identity,
                                                     bias=bias_t[c][:, 0:1],
                                                     scale=alpha_bc[:, 0:1])
                            else:
                                nc.vector.tensor_scalar(out_t[:], pss[si][:],
                                                        alpha_bc[:, 0:1],
                                                        bias_t[c][:, 0:1],
                                                        op0=Alu.mult,
                                                        op1=Alu.add)
                            ep_idx += 1
                            nc.sync.dma_start(
                                y_d[b, c * 128:(c + 1) * 128,
                                    r0:r0 + ROWS_PER_STRIP, :],
                                out_t[:])

    _split_waits(nc, 1)
    return nc


def _prep_host_inputs(x, weight, bias, scaling_first_order,
                      scaling_second_order):
    """Layout-only host prep: shard x over batch, relayout/replicate weights."""
    w_t = np.ascontiguousarray(
        weight.transpose(1, 2, 3, 0)).reshape(CIN, NTAP * COUT)
    s1 = np.asarray(scaling_first_order, np.float32).reshape(COUT)
    s2 = np.asarray(scaling_second_order, np.float32).reshape(COUT)
    s1f = np.ascontiguousarray(
        np.broadcast_to(np.tile(s1, NTAP), (128, NTAP * COUT)))
    s2f = np.ascontiguousarray(
        np.broadcast_to(np.tile(s2, NTAP), (128, NTAP * COUT)))
    b_r = np.asarray(bias, np.float32).reshape(COUT, 1)
    x = np.asarray(x, np.float32)
    in_maps = []
    for i in range(N_CORES):
        in_maps.append({
            "x": np.ascontiguousarray(x[i * B_LOC:(i + 1) * B_LOC]),
            "wt": w_t, "s1f": s1f, "s2f": s2f, "bias": b_r,
        })
    return in_maps


def _install_ntff_hook():
    import types
    try:
        import antenv.axon_hooks  # noqa: F401
        return
    except ImportError:
        pass
    from trn_agent_boot.trn_boot import _ntff_profile_via_ctypes
    hook = _ntff_profile_via_ctypes('/opt/axon/libaxon_pjrt.so')
    mod = types.ModuleType('antenv.axon_hooks')
    mod.get_axon_ntff_profile_hook = lambda: hook
    mod.set_axon_ntff_profile_hook = lambda h: None
    sys.modules['antenv.axon_hooks'] = mod


def _run(in_maps, terms=TERMS, trace=False, debug_taps=False):
    from concourse.bass_utils import run_bass_kernel_spmd
    if trace:
        _install_ntff_hook()
        from concourse import bass_utils
        bass_utils.upload_artifacts = lambda tmpdir: f"local:{tmpdir}"
    nc = _build(terms, debug_taps=debug_taps)
    return run_bass_kernel_spmd(nc, in_maps, list(range(N_CORES)),
                                trace=trace)


def kernel(x, weight, bias, scaling_first_order, scaling_second_order):
    in_maps = _prep_host_inputs(x, weight, bias, scaling_first_order,
                                scaling_second_order)
    res = _run(in_maps, TERMS, trace=False)
    return np.concatenate([res.results[i]["y"] for i in range(N_CORES)],
                          axis=0)
